# revision 25
# baseline (speedup 1.0000x reference)
"""Expert-parallel MoE MLP (BaseMLPExperts) for 8 TRN2 NeuronCores.

Reference computation (per expert e):
    y[:, e, :] = gelu_exact(x[:, e, :] @ wi[e]) @ wo[e]
with T=8192 tokens, E=8 experts, H=1024 hidden, I=4096 intermediate, fp32.

Sharding: expert-parallel — core e owns expert e (its x slice, wi[e], wo[e]).
No cross-core communication.

Per-core device kernel — fully fused, bf16 matmul inputs (PE runs bf16 at
the same 1 column/cycle rate as f32r, and bf16's fast weight load keeps the
LDW+MM pair at the 216ns/MM floor vs f32r's 227ns; measured end-to-end
rel-err ~3.4e-3 vs the 2e-2 gate):

  Both weights live in SBUF for the whole kernel (wi 64KB/partition +
  wo 64KB/partition as bf16). Per 512-token tile (16 tiles):
    GEMM1: for each of 32 i-blocks, 8 accumulating matmuls
           (wi block stationary, x tile moving) -> PSUM [128, 512];
           ACT engine applies exact-erf GELU on eviction, writing bf16
           h1 [128 (i), 32*512 (i-blk, t)] into SBUF (32KB/partition).
    GEMM2: for each of 4 128-token sub-blocks, 2 PSUM banks accumulate
           32 matmuls each (h1 block stationary, wo block moving);
           DVE evacuates y halves f32 to SBUF, DMA per half to DRAM.
  h1 never touches DRAM; the PE sees one uninterrupted matmul stream.

  All device-side loads are host-pre-tiled into consumption order so each
  DMA moves one fat contiguous segment per partition (128 descriptors —
  cheap to kick, full HBM bandwidth). Priming rides the sync + gpsimd
  queues (plus exactly one kick on scalar, whose ring must never back up
  into the gelus), each queue FIFO-ordered by consumption deadline so no
  low-priority bytes (wo) ever sit ahead of wi/xt; ~140 tiny warm-up
  matmuls bridge the priming window so the PE clock gate (HAM) is
  released before the real stream starts.

Host side: pre-tiles x/wi/wo into the consumption-order bf16 layouts,
runs the SPMD kernel on cores 0-7, stacks per-core y into [T, E, H].
"""

import ml_dtypes
import numpy as np

import concourse.bass as bass
import concourse.mybir as mybir
import concourse.tile as tile
from concourse import bacc
from concourse.bass_utils import run_bass_kernel_spmd

T, E, H, I = 8192, 8, 1024, 4096
P = 128
F32 = mybir.dt.float32
BF16 = mybir.dt.bfloat16

TT = 512             # token tile
NT = T // TT         # 16
HT = H // P          # 8 k-tiles for GEMM1
IT = I // P          # 32 i-tiles
TB = 128             # GEMM2 token sub-block
NB = TT // TB        # 4

# run_bass_kernel_spmd kwargs injected by test harness (e.g. trace=True)
RUN_KWARGS: dict = {}
LAST_RESULT = None

_NC = None


def _build():
    nc = bacc.Bacc("TRN2", target_bir_lowering=False, debug=False, num_devices=8)

    # Host-pre-tiled inputs (see kernel() for the layouts):
    #   xd[tt*P + p, ho*TT + t] = x[tt*TT + t, ho*P + p]
    #   wid[k*P + p, ho*P + i]  = wi[ho*P + p, k*P + i]
    #   wod[p, io*H + h]        = wo[io*P + p, h]
    xd = nc.dram_tensor("xd", [NT * P, HT * TT], BF16, kind="ExternalInput").ap()
    wid = nc.dram_tensor("wid", [IT * P, HT * P], BF16, kind="ExternalInput").ap()
    wod = nc.dram_tensor("wod", [P, IT * H], BF16, kind="ExternalInput").ap()
    y = nc.dram_tensor("y", [T, H], F32, kind="ExternalOutput").ap()

    with tile.TileContext(nc) as tc:
        w_pool = tc.alloc_tile_pool(name="w_pool", bufs=1)
        wi_sb = w_pool.tile([P, IT * HT * P], BF16, name="wi_sb")   # 64KB/part
        wo_sb = w_pool.tile([P, IT * H], BF16, name="wo_sb")        # 64KB/part

        with (
            tc.tile_pool(name="xt_pool", bufs=2) as xt_pool,
            tc.tile_pool(name="h1_pool", bufs=1) as h1_pool,
            tc.tile_pool(name="yo_pool", bufs=3) as yo_pool,
            tc.tile_pool(name="ps1_pool", bufs=4, space="PSUM") as ps1_pool,
            tc.tile_pool(name="ps2_pool", bufs=4, space="PSUM") as ps2_pool,
        ):
            def load_xt(tt, engs=(nc.sync, nc.scalar)):
                xt = xt_pool.tile([P, HT * TT], BF16, name="xt", tag="xt")
                n = (HT * TT) // len(engs)
                for g, eng in enumerate(engs):
                    eng.dma_start(
                        out=xt[:, n * g : n * (g + 1)],
                        in_=xd[tt * P : (tt + 1) * P, n * g : n * (g + 1)],
                    )
                return xt

            # ---- priming ----
            # Only sync + gpsimd issue priming DMAs: the kick instruction
            # BLOCKS when its queue ring (depth ~4) is full, and a blocked
            # kick on the ACT engine would sit ahead of the first GELUs in
            # ACT's instruction stream, stalling GEMM1 via ps1-slot reuse.
            # Each queue's FIFO is ordered by consumption deadline; gpsimd
            # (software DGE, observed ~2x the HWDGE share) carries the
            # first-matmul critical set and the odd wi pieces.
            def prime_wi(eng, j):  # 512KB piece: i-chunks 2j, 2j+1
                c0, c1 = 2 * j * P, (2 * j + 2) * P
                dst = wi_sb[:, 2 * j * (HT * P) : (2 * j + 2) * (HT * P)]
                eng.dma_start(
                    out=dst.rearrange("p (a f) -> p a f", a=2),
                    in_=wid[c0:c1, :].rearrange("(a p) f -> p a f", p=P),
                )

            def prime_wo(eng, g):  # 1MB piece: io-blocks 4g..4g+3
                eng.dma_start(
                    out=wo_sb[:, g * 4096 : (g + 1) * 4096],
                    in_=wod[:, g * 4096 : (g + 1) * 4096],
                )

            def prime_xt(eng, tt, half, xt):
                eng.dma_start(
                    out=xt[:, 2048 * half : 2048 * (half + 1)],
                    in_=xd[tt * P : (tt + 1) * P, 2048 * half : 2048 * (half + 1)],
                )

            xt_cur = xt_pool.tile([P, HT * TT], BF16, name="xt", tag="xt")
            xt_nxt = xt_pool.tile([P, HT * TT], BF16, name="xt", tag="xt")
            # first-MM critical set (xt0 + first wi i-chunk) avoids the
            # gpsimd queue, whose first packet lags its kick by ~4us; the
            # scalar (ACT) queue ramps fastest but gets exactly 3 kicks —
            # its ring must never back up into the gelus
            nc.scalar.dma_start(out=wi_sb[:, 0 : HT * P], in_=wid[0:P, :])
            prime_xt(nc.scalar, 0, 0, xt_cur)
            nc.sync.dma_start(out=xt_cur[:, 2048:3072], in_=xd[0:P, 2048:3072])
            nc.gpsimd.dma_start(out=xt_cur[:, 3072:4096], in_=xd[0:P, 3072:4096])
            nc.sync.dma_start(
                out=wi_sb[:, HT * P : 2 * HT * P], in_=wid[P : 2 * P, :]
            )
            prime_wi(nc.scalar, 1)
            for j in range(2, 16):
                prime_wi(nc.gpsimd if j % 2 == 0 else nc.sync, j)
            for g in range(8):
                # the slower sync queue gets only one wo piece; everything
                # is FIFO-ordered by consumption deadline within its queue
                prime_wo(nc.sync if g == 3 else nc.gpsimd, g)
            prime_xt(nc.gpsimd, 1, 0, xt_nxt)
            prime_xt(nc.sync, 1, 1, xt_nxt)

            # HAM pre-warm: tiny matmuls on scratch data bridging the whole
            # priming window (~7.5us..~14.5us), so the PE's clock gate has
            # released (>=3.4us of sustained activity -> K=8/8, 2.4GHz) and
            # never re-arms before the real stream starts; also ramps power
            # gradually instead of spiking from idle.
            with tc.tile_pool(name="warm_pool", bufs=1) as warm_pool:
                warm_sb = warm_pool.tile([P, 64], BF16, name="warm_sb")
                nc.vector.memset(warm_sb[:], 0.0)
                warm_ps = ps1_pool.tile([P, 64], F32, name="ps1", tag="ps1")
                for _ in range(104):
                    nc.tensor.matmul(
                        warm_ps[0:64, :], warm_sb[:], warm_sb[:],
                        start=True, stop=True,
                    )

            # ---- main loop ----
            for tt in range(NT):
                # GEMM1 + GELU: h1 = gelu(x @ wi), i-blocks on partitions
                h1 = h1_pool.tile([P, IT * TT], BF16, name="h1", tag="h1")
                for i in range(IT):
                    ps = ps1_pool.tile([P, TT], F32, name="ps1", tag="ps1")
                    for h in range(HT):
                        nc.tensor.matmul(
                            ps[:],
                            wi_sb[:, i * (HT * P) + h * P : i * (HT * P) + (h + 1) * P],
                            xt_cur[:, h * TT : (h + 1) * TT],
                            start=(h == 0),
                            stop=(h == HT - 1),
                        )
                    nc.scalar.activation(
                        h1[:, i * TT : (i + 1) * TT],
                        ps[:],
                        mybir.ActivationFunctionType.Gelu,
                    )
                if tt + 2 < NT:
                    xt_pre = load_xt(tt + 2, engs=(nc.sync, nc.gpsimd))

                # GEMM2: y = h1 @ wo
                for tb in range(NB):
                    t0 = tt * TT + tb * TB
                    pss = [
                        ps2_pool.tile([P, 512], F32, name="ps2", tag="ps2")
                        for _ in range(2)
                    ]
                    last = tt == NT - 1 and tb == NB - 1
                    if last:
                        # two passes (hh=1 fully first) so the hh=1 copy and
                        # store overlap the final 32 matmuls
                        order = [(i, hh) for hh in (1, 0) for i in range(IT)]
                    else:
                        order = [(i, hh) for i in range(IT) for hh in range(2)]
                    yo = yo_pool.tile([P, H], F32, name="yo", tag="yo")
                    for i, hh in order:
                        nc.tensor.matmul(
                            pss[hh][:],
                            h1[:, i * TT + tb * TB : i * TT + (tb + 1) * TB],
                            wo_sb[:, i * H + hh * 512 : i * H + (hh + 1) * 512],
                            start=(i == 0),
                            stop=(i == IT - 1),
                        )
                        if last and hh == 1 and i == IT - 1:
                            nc.vector.tensor_copy(yo[:, 512:1024], pss[1][:])
                            nc.scalar.dma_start(
                                out=y[t0 : t0 + TB, 512:1024], in_=yo[:, 512:1024]
                            )
                    if last:
                        # quarter-granularity: first store kicks one DVE
                        # copy earlier, both stores run on parallel queues
                        nc.vector.tensor_copy(yo[:, 0:256], pss[0][:, 0:256])
                        nc.sync.dma_start(
                            out=y[t0 : t0 + TB, 0:256], in_=yo[:, 0:256]
                        )
                        nc.vector.tensor_copy(yo[:, 256:512], pss[0][:, 256:512])
                        nc.scalar.dma_start(
                            out=y[t0 : t0 + TB, 256:512], in_=yo[:, 256:512]
                        )
                    else:
                        for hh, eng in ((0, nc.sync), (1, nc.scalar)):
                            nc.vector.tensor_copy(
                                yo[:, hh * 512 : (hh + 1) * 512], pss[hh][:]
                            )
                            eng.dma_start(
                                out=y[t0 : t0 + TB, hh * 512 : (hh + 1) * 512],
                                in_=yo[:, hh * 512 : (hh + 1) * 512],
                            )
                if tt + 2 < NT:
                    xt_cur, xt_nxt = xt_nxt, xt_pre
                else:
                    xt_cur = xt_nxt
        w_pool.release()

    nc.compile()
    return nc


def kernel(x: np.ndarray, wi: np.ndarray, wo: np.ndarray) -> np.ndarray:
    global _NC, LAST_RESULT
    x = np.asarray(x, dtype=np.float32)
    wi = np.asarray(wi, dtype=np.float32)
    wo = np.asarray(wo, dtype=np.float32)
    assert x.shape == (T, E, H) and wi.shape == (E, H, I) and wo.shape == (E, I, H)

    if _NC is None:
        _NC = _build()

    bf = ml_dtypes.bfloat16
    in_maps = []
    for e in range(E):
        # xd[tt*P + p, ho*TT + t] = x[tt*TT + t, ho*P + p]
        xd = (
            x[:, e, :]
            .astype(bf)
            .reshape(NT, TT, HT, P)
            .transpose(0, 3, 2, 1)
            .reshape(NT * P, HT * TT)
        )
        # wid[k*P + p, ho*P + i] = wi[ho*P + p, k*P + i]
        wid = (
            wi[e]
            .astype(bf)
            .reshape(HT, P, IT, P)
            .transpose(2, 1, 0, 3)
            .reshape(IT * P, HT * P)
        )
        # wod[p, io*H + h] = wo[io*P + p, h]
        wod = (
            wo[e]
            .astype(bf)
            .reshape(IT, P, H)
            .transpose(1, 0, 2)
            .reshape(P, IT * H)
        )
        in_maps.append(
            {
                "xd": np.ascontiguousarray(xd),
                "wid": np.ascontiguousarray(wid),
                "wod": np.ascontiguousarray(wod),
            }
        )
    try:
        res = run_bass_kernel_spmd(
            _NC, in_maps, core_ids=list(range(E)), **RUN_KWARGS
        )
    except Exception:
        res = run_bass_kernel_spmd(
            _NC, in_maps, core_ids=list(range(E)), **RUN_KWARGS
        )
    LAST_RESULT = res
    out = np.stack([res.results[e]["y"] for e in range(E)], axis=1)
    return np.ascontiguousarray(out.astype(np.float32, copy=False))


# revision 27
# speedup vs baseline: 1.0010x; 1.0010x over previous
"""Expert-parallel MoE MLP (BaseMLPExperts) for 8 TRN2 NeuronCores.

Reference computation (per expert e):
    y[:, e, :] = gelu_exact(x[:, e, :] @ wi[e]) @ wo[e]
with T=8192 tokens, E=8 experts, H=1024 hidden, I=4096 intermediate, fp32.

Sharding: expert-parallel — core e owns expert e (its x slice, wi[e], wo[e]).
No cross-core communication.

Per-core device kernel — fully fused, bf16 matmul inputs (PE runs bf16 at
the same 1 column/cycle rate as f32r, and bf16's fast weight load keeps the
LDW+MM pair at the 216ns/MM floor vs f32r's 227ns; measured end-to-end
rel-err ~3.4e-3 vs the 2e-2 gate):

  Both weights live in SBUF for the whole kernel (wi 64KB/partition +
  wo 64KB/partition as bf16). Per 512-token tile (16 tiles):
    GEMM1: for each of 32 i-blocks, 8 accumulating matmuls
           (wi block stationary, x tile moving) -> PSUM [128, 512];
           ACT engine applies exact-erf GELU on eviction, writing bf16
           h1 [128 (i), 32*512 (i-blk, t)] into SBUF (32KB/partition).
    GEMM2: for each of 4 128-token sub-blocks, 2 PSUM banks accumulate
           32 matmuls each (h1 block stationary, wo block moving);
           DVE evacuates y halves f32 to SBUF, DMA per half to DRAM.
  h1 never touches DRAM; the PE sees one uninterrupted matmul stream.

  All device-side loads are host-pre-tiled into consumption order so each
  DMA moves one fat contiguous segment per partition (128 descriptors —
  cheap to kick, full HBM bandwidth). Priming rides the sync + gpsimd
  queues (plus exactly one kick on scalar, whose ring must never back up
  into the gelus), each queue FIFO-ordered by consumption deadline so no
  low-priority bytes (wo) ever sit ahead of wi/xt; ~140 tiny warm-up
  matmuls bridge the priming window so the PE clock gate (HAM) is
  released before the real stream starts.

Host side: pre-tiles x/wi/wo into the consumption-order bf16 layouts,
runs the SPMD kernel on cores 0-7, stacks per-core y into [T, E, H].
"""

import ml_dtypes
import numpy as np

import concourse.bass as bass
import concourse.mybir as mybir
import concourse.tile as tile
from concourse import bacc
from concourse.bass_utils import run_bass_kernel_spmd

T, E, H, I = 8192, 8, 1024, 4096
P = 128
F32 = mybir.dt.float32
BF16 = mybir.dt.bfloat16

TT = 512             # token tile
NT = T // TT         # 16
HT = H // P          # 8 k-tiles for GEMM1
IT = I // P          # 32 i-tiles
TB = 128             # GEMM2 token sub-block
NB = TT // TB        # 4

# run_bass_kernel_spmd kwargs injected by test harness (e.g. trace=True)
RUN_KWARGS: dict = {}
LAST_RESULT = None

_NC = None


def _build():
    nc = bacc.Bacc("TRN2", target_bir_lowering=False, debug=False, num_devices=8)

    # Host-pre-tiled inputs (see kernel() for the layouts):
    #   xd[tt*P + p, ho*TT + t] = x[tt*TT + t, ho*P + p]
    #   wid[k*P + p, ho*P + i]  = wi[ho*P + p, k*P + i]
    #   wod[p, io*H + h]        = wo[io*P + p, h]
    xd = nc.dram_tensor("xd", [NT * P, HT * TT], BF16, kind="ExternalInput").ap()
    wid = nc.dram_tensor("wid", [IT * P, HT * P], BF16, kind="ExternalInput").ap()
    wod = nc.dram_tensor("wod", [P, IT * H], BF16, kind="ExternalInput").ap()
    y = nc.dram_tensor("y", [T, H], F32, kind="ExternalOutput").ap()

    with tile.TileContext(nc) as tc:
        w_pool = tc.alloc_tile_pool(name="w_pool", bufs=1)
        wi_sb = w_pool.tile([P, IT * HT * P], BF16, name="wi_sb")   # 64KB/part
        wo_sb = w_pool.tile([P, IT * H], BF16, name="wo_sb")        # 64KB/part

        with (
            tc.tile_pool(name="xt_pool", bufs=2) as xt_pool,
            tc.tile_pool(name="h1_pool", bufs=1) as h1_pool,
            tc.tile_pool(name="yo_pool", bufs=3) as yo_pool,
            tc.tile_pool(name="ps1_pool", bufs=4, space="PSUM") as ps1_pool,
            tc.tile_pool(name="ps2_pool", bufs=4, space="PSUM") as ps2_pool,
        ):
            def load_xt(tt, engs=(nc.sync, nc.scalar)):
                xt = xt_pool.tile([P, HT * TT], BF16, name="xt", tag="xt")
                n = (HT * TT) // len(engs)
                for g, eng in enumerate(engs):
                    eng.dma_start(
                        out=xt[:, n * g : n * (g + 1)],
                        in_=xd[tt * P : (tt + 1) * P, n * g : n * (g + 1)],
                    )
                return xt

            # ---- priming ----
            # Only sync + gpsimd issue priming DMAs: the kick instruction
            # BLOCKS when its queue ring (depth ~4) is full, and a blocked
            # kick on the ACT engine would sit ahead of the first GELUs in
            # ACT's instruction stream, stalling GEMM1 via ps1-slot reuse.
            # Each queue's FIFO is ordered by consumption deadline; gpsimd
            # (software DGE, observed ~2x the HWDGE share) carries the
            # first-matmul critical set and the odd wi pieces.
            def prime_wi(eng, j):  # 512KB piece: i-chunks 2j, 2j+1
                c0, c1 = 2 * j * P, (2 * j + 2) * P
                dst = wi_sb[:, 2 * j * (HT * P) : (2 * j + 2) * (HT * P)]
                eng.dma_start(
                    out=dst.rearrange("p (a f) -> p a f", a=2),
                    in_=wid[c0:c1, :].rearrange("(a p) f -> p a f", p=P),
                )

            def prime_wo(eng, g):  # 1MB piece: io-blocks 4g..4g+3
                eng.dma_start(
                    out=wo_sb[:, g * 4096 : (g + 1) * 4096],
                    in_=wod[:, g * 4096 : (g + 1) * 4096],
                )

            def prime_xt(eng, tt, half, xt):
                eng.dma_start(
                    out=xt[:, 2048 * half : 2048 * (half + 1)],
                    in_=xd[tt * P : (tt + 1) * P, 2048 * half : 2048 * (half + 1)],
                )

            xt_cur = xt_pool.tile([P, HT * TT], BF16, name="xt", tag="xt")
            xt_nxt = xt_pool.tile([P, HT * TT], BF16, name="xt", tag="xt")
            # first-MM critical set (xt0 + first wi i-chunk) avoids the
            # gpsimd queue, whose first packet lags its kick by ~4us; the
            # scalar (ACT) queue ramps fastest but gets exactly 3 kicks —
            # its ring must never back up into the gelus
            nc.gpsimd.dma_start(out=wi_sb[:, 0 : HT * P // 2], in_=wid[0:P, 0 : HT * P // 2])
            nc.sync.dma_start(out=wi_sb[:, HT * P // 2 : HT * P], in_=wid[0:P, HT * P // 2 :])
            prime_xt(nc.scalar, 0, 0, xt_cur)
            nc.gpsimd.dma_start(out=xt_cur[:, 2048:3072], in_=xd[0:P, 2048:3072])
            nc.sync.dma_start(out=xt_cur[:, 3072:4096], in_=xd[0:P, 3072:4096])
            nc.gpsimd.dma_start(
                out=wi_sb[:, HT * P : 2 * HT * P], in_=wid[P : 2 * P, :]
            )
            prime_wi(nc.gpsimd, 1)
            for j in range(2, 16):
                prime_wi(nc.gpsimd if j % 2 == 0 else nc.sync, j)
            for g in range(8):
                # the slower sync queue gets only one wo piece; everything
                # is FIFO-ordered by consumption deadline within its queue
                prime_wo(nc.sync if g == 3 else nc.gpsimd, g)
            prime_xt(nc.gpsimd, 1, 0, xt_nxt)
            prime_xt(nc.sync, 1, 1, xt_nxt)

            # HAM pre-warm: tiny matmuls on scratch data bridging the whole
            # priming window (~7.5us..~14.5us), so the PE's clock gate has
            # released (>=3.4us of sustained activity -> K=8/8, 2.4GHz) and
            # never re-arms before the real stream starts; also ramps power
            # gradually instead of spiking from idle.
            with tc.tile_pool(name="warm_pool", bufs=1) as warm_pool:
                warm_sb = warm_pool.tile([P, 64], BF16, name="warm_sb")
                nc.vector.memset(warm_sb[:], 0.0)
                warm_ps = ps1_pool.tile([P, 64], F32, name="ps1", tag="ps1")
                for _ in range(140):
                    nc.tensor.matmul(
                        warm_ps[0:64, :], warm_sb[:], warm_sb[:],
                        start=True, stop=True,
                    )

            # ---- main loop ----
            for tt in range(NT):
                # GEMM1 + GELU: h1 = gelu(x @ wi), i-blocks on partitions
                h1 = h1_pool.tile([P, IT * TT], BF16, name="h1", tag="h1")
                for i in range(IT):
                    ps = ps1_pool.tile([P, TT], F32, name="ps1", tag="ps1")
                    for h in range(HT):
                        nc.tensor.matmul(
                            ps[:],
                            wi_sb[:, i * (HT * P) + h * P : i * (HT * P) + (h + 1) * P],
                            xt_cur[:, h * TT : (h + 1) * TT],
                            start=(h == 0),
                            stop=(h == HT - 1),
                        )
                    nc.scalar.activation(
                        h1[:, i * TT : (i + 1) * TT],
                        ps[:],
                        mybir.ActivationFunctionType.Gelu,
                    )
                if tt + 2 < NT:
                    xt_pre = load_xt(tt + 2, engs=(nc.sync, nc.gpsimd))

                # GEMM2: y = h1 @ wo
                for tb in range(NB):
                    t0 = tt * TT + tb * TB
                    pss = [
                        ps2_pool.tile([P, 512], F32, name="ps2", tag="ps2")
                        for _ in range(2)
                    ]
                    last = tt == NT - 1 and tb == NB - 1
                    if last:
                        # two passes (hh=1 fully first) so the hh=1 copy and
                        # store overlap the final 32 matmuls
                        order = [(i, hh) for hh in (1, 0) for i in range(IT)]
                    else:
                        order = [(i, hh) for i in range(IT) for hh in range(2)]
                    yo = yo_pool.tile([P, H], F32, name="yo", tag="yo")
                    for i, hh in order:
                        nc.tensor.matmul(
                            pss[hh][:],
                            h1[:, i * TT + tb * TB : i * TT + (tb + 1) * TB],
                            wo_sb[:, i * H + hh * 512 : i * H + (hh + 1) * 512],
                            start=(i == 0),
                            stop=(i == IT - 1),
                        )
                        if last and hh == 1 and i == IT - 1:
                            nc.vector.tensor_copy(yo[:, 512:1024], pss[1][:])
                            nc.scalar.dma_start(
                                out=y[t0 : t0 + TB, 512:1024], in_=yo[:, 512:1024]
                            )
                    if last:
                        # quarter-granularity: first store kicks one DVE
                        # copy earlier, both stores run on parallel queues
                        nc.vector.tensor_copy(yo[:, 0:256], pss[0][:, 0:256])
                        nc.sync.dma_start(
                            out=y[t0 : t0 + TB, 0:256], in_=yo[:, 0:256]
                        )
                        nc.vector.tensor_copy(yo[:, 256:512], pss[0][:, 256:512])
                        nc.scalar.dma_start(
                            out=y[t0 : t0 + TB, 256:512], in_=yo[:, 256:512]
                        )
                    else:
                        for hh, eng in ((0, nc.sync), (1, nc.scalar)):
                            nc.vector.tensor_copy(
                                yo[:, hh * 512 : (hh + 1) * 512], pss[hh][:]
                            )
                            eng.dma_start(
                                out=y[t0 : t0 + TB, hh * 512 : (hh + 1) * 512],
                                in_=yo[:, hh * 512 : (hh + 1) * 512],
                            )
                if tt + 2 < NT:
                    xt_cur, xt_nxt = xt_nxt, xt_pre
                else:
                    xt_cur = xt_nxt
        w_pool.release()

    nc.compile()
    return nc


def kernel(x: np.ndarray, wi: np.ndarray, wo: np.ndarray) -> np.ndarray:
    global _NC, LAST_RESULT
    x = np.asarray(x, dtype=np.float32)
    wi = np.asarray(wi, dtype=np.float32)
    wo = np.asarray(wo, dtype=np.float32)
    assert x.shape == (T, E, H) and wi.shape == (E, H, I) and wo.shape == (E, I, H)

    if _NC is None:
        _NC = _build()

    bf = ml_dtypes.bfloat16
    in_maps = []
    for e in range(E):
        # xd[tt*P + p, ho*TT + t] = x[tt*TT + t, ho*P + p]
        xd = (
            x[:, e, :]
            .astype(bf)
            .reshape(NT, TT, HT, P)
            .transpose(0, 3, 2, 1)
            .reshape(NT * P, HT * TT)
        )
        # wid[k*P + p, ho*P + i] = wi[ho*P + p, k*P + i]
        wid = (
            wi[e]
            .astype(bf)
            .reshape(HT, P, IT, P)
            .transpose(2, 1, 0, 3)
            .reshape(IT * P, HT * P)
        )
        # wod[p, io*H + h] = wo[io*P + p, h]
        wod = (
            wo[e]
            .astype(bf)
            .reshape(IT, P, H)
            .transpose(1, 0, 2)
            .reshape(P, IT * H)
        )
        in_maps.append(
            {
                "xd": np.ascontiguousarray(xd),
                "wid": np.ascontiguousarray(wid),
                "wod": np.ascontiguousarray(wod),
            }
        )
    try:
        res = run_bass_kernel_spmd(
            _NC, in_maps, core_ids=list(range(E)), **RUN_KWARGS
        )
    except Exception:
        res = run_bass_kernel_spmd(
            _NC, in_maps, core_ids=list(range(E)), **RUN_KWARGS
        )
    LAST_RESULT = res
    out = np.stack([res.results[e]["y"] for e in range(E)], axis=1)
    return np.ascontiguousarray(out.astype(np.float32, copy=False))
